# revision 24
# baseline (speedup 1.0000x reference)
"""KNN top-16 kernel for Trainium2 (8 NeuronCores, SPMD).

Problem (hardcoded): p1 (4,8192,3) f32, p2 (4,8192,3) f32, lengths1/2 (4,) i32.
Returns (idx int64 (4,8192,16), dists f32 (4,8192,16)) matching
jax.lax.top_k(-sq_dists, 16) semantics with PyTorch3D-style padding.

Sharding: core c handles batch n=c//2, query rows [(c%2)*4096, (c%2+1)*4096).
p2 of that batch is replicated to the core (per the sharding hint).

Device algorithm per 128-query tile (v2 — chunked two-scan selection):
  One fp16 matmul per 512-target chunk computes
    s[i,j] = 2*p1_i.p2_j - ||p2_j||^2 - 60000*(j >= len2)
  via a hi/lo fp16 split (12 contraction rows: ah.bh + ah.bl + al.bh
  - q2h - q2l - mask), accurate to ~3e-5 abs — top-16 selection then
  differs from fp32 only on near-ties (~28 of 512K indices).
  DVE reads each PSUM 1024-wide selection cell (2 banks) directly:
    MAX8 -> per-cell top-8 values  (candV [128,64])
    FIND_INDEX8 -> per-cell relative indices (candI [128,64] u16)
  Both candidate arrays ship to host per tile (the DMA engines have
  ~10x slack); the host merges 64 candidates/query to top-16 with a
  stable argsort, reproducing the device tie order. The DVE thus runs
  ONLY the two irreducible scans per cell. Exactness: the global
  top-16 is covered unless >8 of it falls in one 1024-cell; with
  fp16-split tie noise included, 69 of 512K indices differ from the
  fp32 reference (idx rel-err 9.2e-3, within the 2e-2 gate). Host:
  idx = (slot>>3)*1024 + candI[slot], dists = ||p1||^2 - v.

All per-core inputs are packed into one DRAM tensor (one DMA, one
semaphore) because TRN2 instructions support at most 2 sync waits and
Tile does not collapse transitive cross-queue waits.
"""

import numpy as np
from functools import lru_cache

N, P1, P2, D, K = 4, 8192, 8192, 3, 16
N_CORES = 8
QPC = P1 // 2          # queries per core (4096)
TILE = 128             # query rows per tile
NTILES = QPC // TILE   # 32
CHUNK = 512            # matmul free-dim chunk == PSUM bank
CELL = 1024            # selection cell (two PSUM banks per MAX8 scan)
NCELL = P2 // CELL     # 8
NCAND = NCELL * 8      # 64 candidates per query
ROWS = 12              # fp16 contraction rows
MASKV = np.float32(60000.0)
INW = QPC + P2         # packed input width per partition (12288)


@lru_cache(maxsize=1)
def _build_program():
    from concourse.bass import Bass
    from concourse.tile import TileContext
    import concourse.mybir as mybir

    f32 = mybir.dt.float32
    f16 = mybir.dt.float16
    u16 = mybir.dt.uint16

    nc = Bass("TRN2", num_devices=N_CORES)

    u32 = mybir.dt.uint32

    inp_d = nc.dram_tensor("inp", [ROWS, INW], f16, kind="ExternalInput")
    # Fused per-tile candidate block, one DMA per tile: u32 words
    # [0:64] = candV f32 (bitcast), [64:96] = candI u16 pairs.
    # p-major staging layout: [p, t*96+w]; host permutes to [t*128+p, w].
    CW = NCAND + NCAND // 2   # 96 u32 words per tile
    cand_d = nc.dram_tensor("cand_out", [TILE, NTILES * CW], u32,
                            kind="ExternalOutput")

    with TileContext(nc) as tc:
        with tc.tile_pool(name="const", bufs=1) as cpool, \
             tc.tile_pool(name="cand", bufs=2) as spool, \
             tc.tile_pool(name="psum", bufs=4, space="PSUM") as ppool:
            inp_sb = cpool.tile([ROWS, INW], f16)
            # Input DMAs split and ordered by first-need time (same queue =>
            # serial in issue order): tile-0's lhsT, then cells 0-2, then the
            # remaining queries, then cells 3-7. Compute starts ~10us earlier
            # than with one monolithic transfer.
            def span(a, b):
                nc.sync.dma_start(inp_sb[:, a:b], inp_d[:, a:b])
            span(0, TILE)                                  # stat tile 0
            for c in range(3):                             # cells 0-2
                span(QPC + c * CELL, QPC + (c + 1) * CELL)
            span(TILE, QPC)                                # stat tiles 1-31
            for c in range(3, NCELL):                      # cells 3-7
                span(QPC + c * CELL, QPC + (c + 1) * CELL)
            stat_sb = inp_sb[:, 0:QPC]
            mov_sb = inp_sb[:, QPC:INW]

            for t in range(NTILES):
                cand = spool.tile([TILE, CW], u32, tag="cand")
                candV = cand[:, 0:NCAND].bitcast(f32)
                candI = cand[:, NCAND:CW].bitcast(u16)
                # 1-element dummy write absorbs the pool slot-reuse wait
                # (the tile's last reader is the Sync-engine DMA) so the
                # real cell ops each carry only the PE-semaphore wait.
                nc.vector.memset(cand[:, 0:1], 0)
                lhsT = stat_sb[:, t * TILE:(t + 1) * TILE]
                for c in range(NCELL):
                    ps = ppool.tile([TILE, CELL], f32, tag="ps")
                    for half in range(CELL // CHUNK):
                        j0 = c * CELL + half * CHUNK
                        nc.tensor.matmul(
                            ps[:, half * CHUNK:(half + 1) * CHUNK], lhsT,
                            mov_sb[:, j0:j0 + CHUNK],
                            start=True, stop=True,
                        )
                    cv = candV[:, c * 8:(c + 1) * 8]
                    nc.vector.max(out=cv, in_=ps)
                    nc.vector.max_index(
                        out=candI[:, c * 8:(c + 1) * 8], in_max=cv,
                        in_values=ps)

                nc.sync.dma_start(cand_d[:, t * CW:(t + 1) * CW], cand)

    # This walrus build allows only ~1 sync wait per instruction; the
    # framework tail Drain carries one wait per busy proc. Split all but
    # the last wait onto single-wait NoOps chained before it (same engine,
    # program order => identical blocking semantics).
    import concourse.mybir as mb
    fix = 0
    for fn in nc.m.functions:
        for blk in fn.blocks:
            insts = blk.instructions
            i = 0
            while i < len(insts):
                inst = insts[i]
                si = inst.sync_info
                if si is not None and len(si.on_wait) > 1:
                    head, last = si.on_wait[:-1], si.on_wait[-1:]
                    pre = []
                    for w in head:
                        fix += 1
                        nop = mb.InstNoOp(name=f"I-waitfix-{fix}", ins=[],
                                          outs=[])
                        nop.engine = inst.engine
                        nop.sync_info = mb.SyncInfo(on_wait=[w], on_update=[])
                        pre.append(nop)
                    si.on_wait = last
                    insts[i:i] = pre
                    i += len(pre)
                i += 1
    return nc


def _f16(x):
    return np.asarray(x, np.float16)


def _core_inputs(p1, p2, lengths2, core):
    n, h = core // 2, core % 2
    q0 = h * QPC
    a = 2.0 * p1[n, q0:q0 + QPC]       # (4096, 3) f32
    b = p2[n]                           # (8192, 3) f32

    ah = _f16(a)
    al = _f16(a - ah.astype(np.float32))
    bh = _f16(b)
    bl = _f16(b - bh.astype(np.float32))
    q2 = (b.astype(np.float64) ** 2).sum(1).astype(np.float32)
    q2h = _f16(q2)
    q2l = _f16(q2 - q2h.astype(np.float32))
    mask = np.where(np.arange(P2) >= lengths2[n], MASKV, np.float32(0.0))

    inp = np.empty((ROWS, INW), np.float16)
    stat = inp[:, 0:QPC]
    mov = inp[:, QPC:INW]
    stat[0:3] = ah.T
    stat[3:6] = ah.T
    stat[6:9] = al.T
    stat[9:12] = np.float16(-1.0)
    mov[0:3] = bh.T
    mov[3:6] = bl.T
    mov[6:9] = bh.T
    mov[9] = q2h
    mov[10] = q2l
    mov[11] = _f16(mask)
    return {"inp": inp}


def kernel(p1, p2, lengths1, lengths2):
    from concourse.bass_utils import run_bass_kernel_spmd

    p1 = np.asarray(p1, np.float32)
    p2 = np.asarray(p2, np.float32)
    lengths1 = np.asarray(lengths1, np.int32)
    lengths2 = np.asarray(lengths2, np.int32)

    nc = _build_program()
    in_maps = [_core_inputs(p1, p2, lengths2, c) for c in range(N_CORES)]
    res = run_bass_kernel_spmd(nc, in_maps, core_ids=list(range(N_CORES)))

    # host epilogue: dists = ||p1||^2 - s, idx recovery, pad-row zeroing
    p1sq = (p1[:, :, 0] * p1[:, :, 0] + p1[:, :, 1] * p1[:, :, 1]) \
        + p1[:, :, 2] * p1[:, :, 2]                      # (4, 8192) f32

    dists = np.zeros((N, P1, K), np.float32)
    idx = np.zeros((N, P1, K), np.int64)
    CW = NCAND + NCAND // 2
    for c in range(N_CORES):
        n, h = c // 2, c % 2
        sl = slice(h * QPC, (h + 1) * QPC)
        blk = res.results[c]["cand_out"].reshape(TILE, NTILES, CW)
        blk = np.ascontiguousarray(blk.transpose(1, 0, 2))  # (NT, 128, 96)
        cv = blk[:, :, 0:NCAND].view(np.float32).reshape(QPC, NCAND)
        ci = blk[:, :, NCAND:CW].view(np.uint16).reshape(QPC, NCAND)
        # top-16 of 64 candidates; stable sort = device tie order
        # (value desc, then lower slot = lower cell = lower index)
        slot = np.argsort(-cv, axis=1, kind="stable")[:, :K]
        selV = np.take_along_axis(cv, slot, axis=1)
        rel = np.take_along_axis(ci, slot, axis=1).astype(np.int64)
        dists[n, sl] = p1sq[n, sl, None] - selV
        idx[n, sl] = (slot >> 3) * CELL + rel

    for n in range(N):
        L = int(lengths1[n])
        dists[n, L:] = 0.0
        idx[n, L:] = 0
    return idx, dists


# revision 25
# speedup vs baseline: 1.0951x; 1.0951x over previous
"""KNN top-16 kernel for Trainium2 (8 NeuronCores, SPMD).

Problem (hardcoded): p1 (4,8192,3) f32, p2 (4,8192,3) f32, lengths1/2 (4,) i32.
Returns (idx int64 (4,8192,16), dists f32 (4,8192,16)) matching
jax.lax.top_k(-sq_dists, 16) semantics with PyTorch3D-style padding.

Sharding (v3 — cross-batch, mask-aware): core c handles query rows
[1024c, 1024(c+1)) of EVERY batch, and scans only that batch's VALID
1024-wide column cells (C_n = ceil(lengths2[n]/1024)); fully-masked
tail cells are skipped. This both drops ~10% of the scan work and
keeps all 8 cores exactly balanced. The program is built at runtime
from the actual lengths2 (lru-cached on the cell counts).

Device algorithm per 128-query tile (tile t -> batch n = t//8):
  One fp16 matmul per 512-target chunk computes
    s[i,j] = 2*p1_i.p2_j - ||p2_j||^2 - 60000*(j >= len2)
  via a hi/lo fp16 split (12 contraction rows: ah.bh + ah.bl + al.bh
  - q2h - q2l - mask), accurate to ~3e-5 abs.
  DVE reads each PSUM 1024-wide selection cell (2 banks) directly:
    MAX8 -> per-cell top-8 values, FIND_INDEX8 -> relative indices,
  written into one fused u32 candidate block per tile
  (words [0:C*8] = values f32, [C*8:C*12] = indices u16-pairs), which
  ships to host in a single DMA. The host merges C*8 candidates/query
  to top-16 with a stable argsort (reproduces device tie order).
  The DVE thus runs ONLY the two irreducible scans per cell at its
  1 elem/cycle datapath limit. Exactness: the global top-16 is
  covered unless >8 of it falls in one 1024-cell; with fp16-split tie
  noise included, 69 of 512K indices differ from the fp32 reference
  (idx rel-err 9.2e-3, within the 2e-2 gate). Host:
  idx = (slot>>3)*1024 + rel, dists = ||p1||^2 - v.

All per-core inputs are packed into one DRAM tensor, transferred by a
few need-ordered DMAs (TRN2 instructions support at most 2 sync waits
and Tile does not collapse transitive cross-queue waits; the tail
fixup splits any multi-wait onto single-wait NoOps).
"""

import numpy as np
from functools import lru_cache

N, P1, P2, D, K = 4, 8192, 8192, 3, 16
N_CORES = 8
QPB = P1 // N_CORES    # queries per (core, batch) = 1024
QPC = QPB * N          # queries per core (4096)
TILE = 128             # query rows per tile
NTILES = QPC // TILE   # 32 (8 per batch)
TPB = QPB // TILE      # tiles per batch per core (8)
CHUNK = 512            # matmul free-dim chunk == PSUM bank
CELL = 1024            # selection cell (two PSUM banks per MAX8 scan)
ROWS = 12              # fp16 contraction rows
MASKV = np.float32(60000.0)


def _cells(lengths2):
    return tuple(int(-(-int(l) // CELL)) for l in lengths2)


@lru_cache(maxsize=2)
def _build_program(cells):
    from concourse.bass import Bass
    from concourse.tile import TileContext
    import concourse.mybir as mybir

    f32 = mybir.dt.float32
    f16 = mybir.dt.float16
    u16 = mybir.dt.uint16
    u32 = mybir.dt.uint32

    movw = CELL * sum(cells)        # mov columns (29696 for seed-0 lengths)
    inw = QPC + movw
    moff = [QPC + CELL * sum(cells[:n]) for n in range(N)]  # per-batch mov base
    # fused per-tile candidate block widths (u32 words) and offsets
    tw = [cells[t // TPB] * 12 for t in range(NTILES)]
    toff = np.concatenate([[0], np.cumsum(tw)]).tolist()

    nc = Bass("TRN2", num_devices=N_CORES)

    inp_d = nc.dram_tensor("inp", [ROWS, inw], f16, kind="ExternalInput")
    cand_d = nc.dram_tensor("cand_out", [TILE, toff[-1]], u32,
                            kind="ExternalOutput")

    with TileContext(nc) as tc:
        with tc.tile_pool(name="const", bufs=1) as cpool, \
             tc.tile_pool(name="cand", bufs=2) as spool, \
             tc.tile_pool(name="psum", bufs=4, space="PSUM") as ppool:
            inp_sb = cpool.tile([ROWS, inw], f16)
            # Input DMAs split and ordered by first-need time (same queue =>
            # serial in issue order): tile-0's lhsT, then batch-0 cells 0-2,
            # then the remaining queries, then the rest of the columns.
            def span(a, b):
                nc.sync.dma_start(inp_sb[:, a:b], inp_d[:, a:b])
            span(0, TILE)                                  # stat tile 0
            span(QPC, QPC + 3 * CELL)                      # cells 0-2
            span(TILE, QPC)                                # stat tiles 1-31
            span(QPC + 3 * CELL, inw)                      # remaining cells
            stat_sb = inp_sb[:, 0:QPC]

            for t in range(NTILES):
                C = cells[t // TPB]
                cand = spool.tile([TILE, C * 12], u32, tag="cand")
                candV = cand[:, 0:C * 8].bitcast(f32)
                candI = cand[:, C * 8:C * 12].bitcast(u16)
                # 1-element dummy write absorbs the pool slot-reuse wait
                # (the tile's last reader is the Sync-engine DMA) so the
                # real cell ops each carry only the PE-semaphore wait.
                nc.vector.memset(cand[:, 0:1], 0)
                lhsT = stat_sb[:, t * TILE:(t + 1) * TILE]
                base = moff[t // TPB]
                for c in range(C):
                    ps = ppool.tile([TILE, CELL], f32, tag="ps")
                    for half in range(CELL // CHUNK):
                        j0 = base + c * CELL + half * CHUNK
                        nc.tensor.matmul(
                            ps[:, half * CHUNK:(half + 1) * CHUNK], lhsT,
                            inp_sb[:, j0:j0 + CHUNK],
                            start=True, stop=True,
                        )
                    cv = candV[:, c * 8:(c + 1) * 8]
                    nc.vector.max(out=cv, in_=ps)
                    nc.vector.max_index(
                        out=candI[:, c * 8:(c + 1) * 8], in_max=cv,
                        in_values=ps)
                nc.sync.dma_start(cand_d[:, toff[t]:toff[t + 1]], cand)

    # This walrus build allows only ~1 sync wait per instruction; the
    # framework tail Drain carries one wait per busy proc. Split all but
    # the last wait onto single-wait NoOps chained before it (same engine,
    # program order => identical blocking semantics).
    import concourse.mybir as mb
    fix = 0
    for fn in nc.m.functions:
        for blk in fn.blocks:
            insts = blk.instructions
            i = 0
            while i < len(insts):
                inst = insts[i]
                si = inst.sync_info
                if si is not None and len(si.on_wait) > 1:
                    head, last = si.on_wait[:-1], si.on_wait[-1:]
                    pre = []
                    for w in head:
                        fix += 1
                        nop = mb.InstNoOp(name=f"I-waitfix-{fix}", ins=[],
                                          outs=[])
                        nop.engine = inst.engine
                        nop.sync_info = mb.SyncInfo(on_wait=[w], on_update=[])
                        pre.append(nop)
                    si.on_wait = last
                    insts[i:i] = pre
                    i += len(pre)
                i += 1
    return nc


def _f16(x):
    return np.asarray(x, np.float16)


def _core_inputs(p1, p2, lengths2, core):
    cells = _cells(lengths2)
    movw = CELL * sum(cells)
    inw = QPC + movw
    inp = np.empty((ROWS, inw), np.float16)
    q0 = core * QPB

    for n in range(N):
        a = 2.0 * p1[n, q0:q0 + QPB]       # (1024, 3) f32
        ah = _f16(a)
        al = _f16(a - ah.astype(np.float32))
        sl = slice(n * QPB, (n + 1) * QPB)
        inp[0:3, sl] = ah.T
        inp[3:6, sl] = ah.T
        inp[6:9, sl] = al.T
        inp[9:12, sl] = np.float16(-1.0)

        w = cells[n] * CELL
        b = p2[n, 0:w]                      # (w, 3) f32
        bh = _f16(b)
        bl = _f16(b - bh.astype(np.float32))
        q2 = (b.astype(np.float64) ** 2).sum(1).astype(np.float32)
        q2h = _f16(q2)
        q2l = _f16(q2 - q2h.astype(np.float32))
        mask = np.where(np.arange(w) >= lengths2[n], MASKV, np.float32(0.0))
        off = QPC + CELL * sum(cells[:n])
        ms = slice(off, off + w)
        inp[0:3, ms] = bh.T
        inp[3:6, ms] = bl.T
        inp[6:9, ms] = bh.T
        inp[9, ms] = q2h
        inp[10, ms] = q2l
        inp[11, ms] = _f16(mask)
    return {"inp": inp}


def kernel(p1, p2, lengths1, lengths2):
    from concourse.bass_utils import run_bass_kernel_spmd

    p1 = np.asarray(p1, np.float32)
    p2 = np.asarray(p2, np.float32)
    lengths1 = np.asarray(lengths1, np.int32)
    lengths2 = np.asarray(lengths2, np.int32)

    cells = _cells(lengths2)
    nc = _build_program(cells)
    in_maps = [_core_inputs(p1, p2, lengths2, c) for c in range(N_CORES)]
    res = run_bass_kernel_spmd(nc, in_maps, core_ids=list(range(N_CORES)))

    # host epilogue: merge candidates, dists = ||p1||^2 - s, pad zeroing
    p1sq = (p1[:, :, 0] * p1[:, :, 0] + p1[:, :, 1] * p1[:, :, 1]) \
        + p1[:, :, 2] * p1[:, :, 2]                      # (4, 8192) f32

    tw = [cells[t // TPB] * 12 for t in range(NTILES)]
    toff = np.concatenate([[0], np.cumsum(tw)]).astype(int)

    dists = np.zeros((N, P1, K), np.float32)
    idx = np.zeros((N, P1, K), np.int64)
    for core in range(N_CORES):
        out = res.results[core]["cand_out"]              # (128, toff[-1]) u32
        for t in range(NTILES):
            n, b = t // TPB, t % TPB
            C = cells[n]
            blk = np.ascontiguousarray(out[:, toff[t]:toff[t + 1]])
            cv = blk[:, 0:C * 8].view(np.float32)        # (128, C*8)
            ci = blk[:, C * 8:C * 12].view(np.uint16)    # (128, C*8)
            # top-16 of C*8 candidates; stable sort = device tie order
            # (value desc, then lower slot = lower cell = lower index)
            slot = np.argsort(-cv, axis=1, kind="stable")[:, :K]
            selV = np.take_along_axis(cv, slot, axis=1)
            rel = np.take_along_axis(ci, slot, axis=1).astype(np.int64)
            rows = slice(core * QPB + b * TILE, core * QPB + (b + 1) * TILE)
            dists[n, rows] = p1sq[n, rows, None] - selV
            idx[n, rows] = (slot >> 3) * CELL + rel

    for n in range(N):
        L = int(lengths1[n])
        dists[n, L:] = 0.0
        idx[n, L:] = 0
    return idx, dists


# revision 26
# speedup vs baseline: 1.0987x; 1.0032x over previous
"""KNN top-16 kernel for Trainium2 (8 NeuronCores, SPMD).

Problem (hardcoded): p1 (4,8192,3) f32, p2 (4,8192,3) f32, lengths1/2 (4,) i32.
Returns (idx int64 (4,8192,16), dists f32 (4,8192,16)) matching
jax.lax.top_k(-sq_dists, 16) semantics with PyTorch3D-style padding.

Sharding (v3 — cross-batch, mask-aware): core c handles query rows
[1024c, 1024(c+1)) of EVERY batch, and scans only that batch's VALID
1024-wide column cells (C_n = ceil(lengths2[n]/1024)); fully-masked
tail cells are skipped. This both drops ~10% of the scan work and
keeps all 8 cores exactly balanced. The program is built at runtime
from the actual lengths2 (lru-cached on the cell counts).

Device algorithm per 128-query tile (tile t -> batch n = t//8):
  One fp16 matmul per 512-target chunk computes
    s[i,j] = 2*p1_i.p2_j - ||p2_j||^2 - 60000*(j >= len2)
  via a hi/lo fp16 split (12 contraction rows: ah.bh + ah.bl + al.bh
  - q2h - q2l - mask), accurate to ~3e-5 abs.
  DVE reads each PSUM 1024-wide selection cell (2 banks) directly:
    MAX8 -> per-cell top-8 values, FIND_INDEX8 -> relative indices,
  written into one fused u32 candidate block per tile
  (words [0:C*8] = values f32, [C*8:C*12] = indices u16-pairs), which
  ships to host in a single DMA. The host merges C*8 candidates/query
  to top-16 with a stable argsort (reproduces device tie order).
  The DVE thus runs ONLY the two irreducible scans per cell at its
  1 elem/cycle datapath limit. Exactness: the global top-16 is
  covered unless >8 of it falls in one 1024-cell; with fp16-split tie
  noise included, 69 of 512K indices differ from the fp32 reference
  (idx rel-err 9.2e-3, within the 2e-2 gate). Host:
  idx = (slot>>3)*1024 + rel, dists = ||p1||^2 - v.

All per-core inputs are packed into one DRAM tensor, transferred by a
few need-ordered DMAs (TRN2 instructions support at most 2 sync waits
and Tile does not collapse transitive cross-queue waits; the tail
fixup splits any multi-wait onto single-wait NoOps).
"""

import numpy as np
from functools import lru_cache

N, P1, P2, D, K = 4, 8192, 8192, 3, 16
N_CORES = 8
QPB = P1 // N_CORES    # queries per (core, batch) = 1024
QPC = QPB * N          # queries per core (4096)
TILE = 128             # query rows per tile
NTILES = QPC // TILE   # 32 (8 per batch)
TPB = QPB // TILE      # tiles per batch per core (8)
CHUNK = 512            # matmul free-dim chunk == PSUM bank
CELL = 1024            # selection cell (two PSUM banks per MAX8 scan)
ROWS = 12              # fp16 contraction rows
MASKV = np.float32(60000.0)


def _cells(lengths2):
    return tuple(int(-(-int(l) // CELL)) for l in lengths2)


@lru_cache(maxsize=2)
def _build_program(cells):
    from concourse.bass import Bass
    from concourse.tile import TileContext
    import concourse.mybir as mybir

    f32 = mybir.dt.float32
    f16 = mybir.dt.float16
    u16 = mybir.dt.uint16
    u32 = mybir.dt.uint32

    movw = CELL * sum(cells)        # mov columns (29696 for seed-0 lengths)
    inw = QPC + movw
    moff = [QPC + CELL * sum(cells[:n]) for n in range(N)]  # per-batch mov base
    # fused per-tile candidate block widths (u32 words) and offsets
    tw = [cells[t // TPB] * 12 for t in range(NTILES)]
    toff = np.concatenate([[0], np.cumsum(tw)]).tolist()

    nc = Bass("TRN2", num_devices=N_CORES)

    inp_d = nc.dram_tensor("inp", [ROWS, inw], f16, kind="ExternalInput")
    cand_d = nc.dram_tensor("cand_out", [TILE, toff[-1]], u32,
                            kind="ExternalOutput")

    with TileContext(nc) as tc:
        with tc.tile_pool(name="const", bufs=1) as cpool, \
             tc.tile_pool(name="cand", bufs=2) as spool, \
             tc.tile_pool(name="psum", bufs=4, space="PSUM") as ppool:
            inp_sb = cpool.tile([ROWS, inw], f16)
            # Input DMAs split and ordered by first-need time (same queue =>
            # serial in issue order): tile-0's lhsT, then batch-0 cells 0-2,
            # then the remaining queries, then the rest of the columns.
            def span(a, b):
                nc.sync.dma_start(inp_sb[:, a:b], inp_d[:, a:b])
            span(0, TILE)                                  # stat tile 0
            span(QPC, QPC + CELL)                          # batch-0 cell 0
            span(QPC + CELL, moff[1])                      # batch-0 cells 1+
            span(TILE, QPC)                                # stat tiles 1-31
            span(moff[1], inw)                             # batches 1-3 cells
            stat_sb = inp_sb[:, 0:QPC]

            for t in range(NTILES):
                C = cells[t // TPB]
                cand = spool.tile([TILE, C * 12], u32, tag="cand")
                candV = cand[:, 0:C * 8].bitcast(f32)
                candI = cand[:, C * 8:C * 12].bitcast(u16)
                # 1-element dummy write absorbs the pool slot-reuse wait
                # (the tile's last reader is the Sync-engine DMA) so the
                # real cell ops each carry only the PE-semaphore wait.
                nc.vector.memset(cand[:, 0:1], 0)
                lhsT = stat_sb[:, t * TILE:(t + 1) * TILE]
                base = moff[t // TPB]
                for c in range(C):
                    ps = ppool.tile([TILE, CELL], f32, tag="ps")
                    for half in range(CELL // CHUNK):
                        j0 = base + c * CELL + half * CHUNK
                        nc.tensor.matmul(
                            ps[:, half * CHUNK:(half + 1) * CHUNK], lhsT,
                            inp_sb[:, j0:j0 + CHUNK],
                            start=True, stop=True,
                        )
                    cv = candV[:, c * 8:(c + 1) * 8]
                    nc.vector.max(out=cv, in_=ps)
                    nc.vector.max_index(
                        out=candI[:, c * 8:(c + 1) * 8], in_max=cv,
                        in_values=ps)
                nc.sync.dma_start(cand_d[:, toff[t]:toff[t + 1]], cand)

    # This walrus build allows only ~1 sync wait per instruction; the
    # framework tail Drain carries one wait per busy proc. Split all but
    # the last wait onto single-wait NoOps chained before it (same engine,
    # program order => identical blocking semantics).
    import concourse.mybir as mb
    fix = 0
    for fn in nc.m.functions:
        for blk in fn.blocks:
            insts = blk.instructions
            i = 0
            while i < len(insts):
                inst = insts[i]
                si = inst.sync_info
                if si is not None and len(si.on_wait) > 1:
                    head, last = si.on_wait[:-1], si.on_wait[-1:]
                    pre = []
                    for w in head:
                        fix += 1
                        nop = mb.InstNoOp(name=f"I-waitfix-{fix}", ins=[],
                                          outs=[])
                        nop.engine = inst.engine
                        nop.sync_info = mb.SyncInfo(on_wait=[w], on_update=[])
                        pre.append(nop)
                    si.on_wait = last
                    insts[i:i] = pre
                    i += len(pre)
                i += 1
    return nc


def _f16(x):
    return np.asarray(x, np.float16)


def _core_inputs(p1, p2, lengths2, core):
    cells = _cells(lengths2)
    movw = CELL * sum(cells)
    inw = QPC + movw
    inp = np.empty((ROWS, inw), np.float16)
    q0 = core * QPB

    for n in range(N):
        a = 2.0 * p1[n, q0:q0 + QPB]       # (1024, 3) f32
        ah = _f16(a)
        al = _f16(a - ah.astype(np.float32))
        sl = slice(n * QPB, (n + 1) * QPB)
        inp[0:3, sl] = ah.T
        inp[3:6, sl] = ah.T
        inp[6:9, sl] = al.T
        inp[9:12, sl] = np.float16(-1.0)

        w = cells[n] * CELL
        b = p2[n, 0:w]                      # (w, 3) f32
        bh = _f16(b)
        bl = _f16(b - bh.astype(np.float32))
        q2 = (b.astype(np.float64) ** 2).sum(1).astype(np.float32)
        q2h = _f16(q2)
        q2l = _f16(q2 - q2h.astype(np.float32))
        mask = np.where(np.arange(w) >= lengths2[n], MASKV, np.float32(0.0))
        off = QPC + CELL * sum(cells[:n])
        ms = slice(off, off + w)
        inp[0:3, ms] = bh.T
        inp[3:6, ms] = bl.T
        inp[6:9, ms] = bh.T
        inp[9, ms] = q2h
        inp[10, ms] = q2l
        inp[11, ms] = _f16(mask)
    return {"inp": inp}


def kernel(p1, p2, lengths1, lengths2):
    from concourse.bass_utils import run_bass_kernel_spmd

    p1 = np.asarray(p1, np.float32)
    p2 = np.asarray(p2, np.float32)
    lengths1 = np.asarray(lengths1, np.int32)
    lengths2 = np.asarray(lengths2, np.int32)

    cells = _cells(lengths2)
    nc = _build_program(cells)
    in_maps = [_core_inputs(p1, p2, lengths2, c) for c in range(N_CORES)]
    res = run_bass_kernel_spmd(nc, in_maps, core_ids=list(range(N_CORES)))

    # host epilogue: merge candidates, dists = ||p1||^2 - s, pad zeroing
    p1sq = (p1[:, :, 0] * p1[:, :, 0] + p1[:, :, 1] * p1[:, :, 1]) \
        + p1[:, :, 2] * p1[:, :, 2]                      # (4, 8192) f32

    tw = [cells[t // TPB] * 12 for t in range(NTILES)]
    toff = np.concatenate([[0], np.cumsum(tw)]).astype(int)

    dists = np.zeros((N, P1, K), np.float32)
    idx = np.zeros((N, P1, K), np.int64)
    for core in range(N_CORES):
        out = res.results[core]["cand_out"]              # (128, toff[-1]) u32
        for t in range(NTILES):
            n, b = t // TPB, t % TPB
            C = cells[n]
            blk = np.ascontiguousarray(out[:, toff[t]:toff[t + 1]])
            cv = blk[:, 0:C * 8].view(np.float32)        # (128, C*8)
            ci = blk[:, C * 8:C * 12].view(np.uint16)    # (128, C*8)
            # top-16 of C*8 candidates; stable sort = device tie order
            # (value desc, then lower slot = lower cell = lower index)
            slot = np.argsort(-cv, axis=1, kind="stable")[:, :K]
            selV = np.take_along_axis(cv, slot, axis=1)
            rel = np.take_along_axis(ci, slot, axis=1).astype(np.int64)
            rows = slice(core * QPB + b * TILE, core * QPB + (b + 1) * TILE)
            dists[n, rows] = p1sq[n, rows, None] - selV
            idx[n, rows] = (slot >> 3) * CELL + rel

    for n in range(N):
        L = int(lengths1[n])
        dists[n, L:] = 0.0
        idx[n, L:] = 0
    return idx, dists


# revision 29
# speedup vs baseline: 1.1487x; 1.0456x over previous
"""KNN top-16 kernel for Trainium2 (8 NeuronCores, SPMD).

Problem (hardcoded): p1 (4,8192,3) f32, p2 (4,8192,3) f32, lengths1/2 (4,) i32.
Returns (idx int64 (4,8192,16), dists f32 (4,8192,16)) matching
jax.lax.top_k(-sq_dists, 16) semantics with PyTorch3D-style padding.

Sharding (v3 — cross-batch, mask-aware): core c handles query rows
[1024c, 1024(c+1)) of EVERY batch, and scans only that batch's VALID
1024-wide column cells (C_n = ceil(lengths2[n]/1024)); fully-masked
tail cells are skipped. This both drops ~10% of the scan work and
keeps all 8 cores exactly balanced. The program is built at runtime
from the actual lengths2 (lru-cached on the cell counts).

Device algorithm per 128-query tile (tile t -> batch n = t//8):
  One fp16 matmul per 512-target chunk computes
    s[i,j] = 2*p1_i.p2_j - ||p2_j||^2 - 60000*(j >= len2)
  via a hi/lo fp16 split (12 contraction rows: ah.bh + ah.bl + al.bh
  - q2h - q2l - mask), accurate to ~3e-5 abs.
  DVE reads each PSUM 1024-wide selection cell (2 banks) directly:
    MAX8 -> per-cell top-8 values, FIND_INDEX8 -> relative indices,
  written into one fused u32 candidate block per tile
  (words [0:C*8] = values f32, [C*8:C*12] = indices u16-pairs), which
  ships to host in a single DMA. The host merges C*8 candidates/query
  to top-16 with a stable argsort (reproduces device tie order).
  The DVE thus runs ONLY the two irreducible scans per cell at its
  1 elem/cycle datapath limit. Exactness: the global top-16 is
  covered unless >8 of it falls in one 1024-cell; with fp16-split tie
  noise included, 69 of 512K indices differ from the fp32 reference
  (idx rel-err 9.2e-3, within the 2e-2 gate). Host:
  idx = (slot>>3)*1024 + rel, dists = ||p1||^2 - v.

All per-core inputs are packed into one DRAM tensor, transferred by a
few need-ordered DMAs (TRN2 instructions support at most 2 sync waits
and Tile does not collapse transitive cross-queue waits; the tail
fixup splits any multi-wait onto single-wait NoOps).
"""

import numpy as np
from functools import lru_cache

N, P1, P2, D, K = 4, 8192, 8192, 3, 16
N_CORES = 8
QPB = P1 // N_CORES    # queries per (core, batch) = 1024
QPC = QPB * N          # queries per core (4096)
TILE = 128             # query rows per tile
NTILES = QPC // TILE   # 32 (8 per batch)
TPB = QPB // TILE      # tiles per batch per core (8)
CHUNK = 512            # matmul free-dim chunk == PSUM bank
CELL = 1024            # selection cell (two PSUM banks per MAX8 scan)
ROWS = 12              # fp16 contraction rows
MASKV = np.float32(60000.0)


def _cells(lengths2):
    return tuple(int(-(-int(l) // CELL)) for l in lengths2)


@lru_cache(maxsize=2)
def _build_program(lengths2):
    from concourse.bass import Bass
    from concourse.tile import TileContext
    import concourse.mybir as mybir

    f32 = mybir.dt.float32
    f16 = mybir.dt.float16
    u16 = mybir.dt.uint16
    u32 = mybir.dt.uint32

    cells = _cells(lengths2)
    # scan width per (batch, cell): full 1024 except the last valid cell,
    # which is trimmed to its valid prefix (>=8 for the MAX8 minimum)
    cw = [[min(CELL, max(8, int(lengths2[n]) - c * CELL))
           for c in range(cells[n])] for n in range(N)]
    movw = CELL * sum(cells)        # mov columns (29696 for seed-0 lengths)
    inw = QPC + movw
    moff = [QPC + CELL * sum(cells[:n]) for n in range(N)]  # per-batch mov base
    # fused per-tile candidate block widths (u32 words) and offsets
    tw = [cells[t // TPB] * 12 for t in range(NTILES)]
    toff = np.concatenate([[0], np.cumsum(tw)]).tolist()

    nc = Bass("TRN2", num_devices=N_CORES)

    inp_d = nc.dram_tensor("inp", [ROWS, inw], f16, kind="ExternalInput")
    cand_d = nc.dram_tensor("cand_out", [TILE, toff[-1]], u32,
                            kind="ExternalOutput")

    with TileContext(nc) as tc:
        with tc.tile_pool(name="const", bufs=1) as cpool, \
             tc.tile_pool(name="cand", bufs=2) as spool, \
             tc.tile_pool(name="psum", bufs=4, space="PSUM") as ppool:
            inp_sb = cpool.tile([ROWS, inw], f16)
            # Input DMAs split and ordered by first-need time (same queue =>
            # serial in issue order): tile-0's lhsT, then batch-0 cells 0-2,
            # then the remaining queries, then the rest of the columns.
            def span(a, b):
                nc.sync.dma_start(inp_sb[:, a:b], inp_d[:, a:b])
            span(0, TILE)                                  # stat tile 0
            span(QPC, QPC + CELL)                          # batch-0 cell 0
            span(QPC + CELL, moff[1])                      # batch-0 cells 1+
            span(TILE, QPC)                                # stat tiles 1-31
            span(moff[1], inw)                             # batches 1-3 cells
            stat_sb = inp_sb[:, 0:QPC]

            for t in range(NTILES):
                C = cells[t // TPB]
                cand = spool.tile([TILE, C * 12], u32, tag="cand")
                candV = cand[:, 0:C * 8].bitcast(f32)
                candI = cand[:, C * 8:C * 12].bitcast(u16)
                # 1-element dummy write absorbs the pool slot-reuse wait
                # (the tile's last reader is the Sync-engine DMA) so the
                # real cell ops each carry only the PE-semaphore wait.
                nc.vector.memset(cand[:, 0:1], 0)
                lhsT = stat_sb[:, t * TILE:(t + 1) * TILE]
                base = moff[t // TPB]
                for c in range(C):
                    w = cw[t // TPB][c]
                    ps = ppool.tile([TILE, CELL], f32, tag="ps")
                    for half in range(-(-w // CHUNK)):
                        j0 = base + c * CELL + half * CHUNK
                        hw = min(CHUNK, w - half * CHUNK)
                        nc.tensor.matmul(
                            ps[:, half * CHUNK:half * CHUNK + hw], lhsT,
                            inp_sb[:, j0:j0 + hw],
                            start=True, stop=True,
                        )
                    cv = candV[:, c * 8:(c + 1) * 8]
                    nc.vector.max(out=cv, in_=ps[:, 0:w])
                    nc.vector.max_index(
                        out=candI[:, c * 8:(c + 1) * 8], in_max=cv,
                        in_values=ps[:, 0:w])
                nc.sync.dma_start(cand_d[:, toff[t]:toff[t + 1]], cand)

    # This walrus build allows only ~1 sync wait per instruction; the
    # framework tail Drain carries one wait per busy proc. Split all but
    # the last wait onto single-wait NoOps chained before it (same engine,
    # program order => identical blocking semantics).
    import concourse.mybir as mb
    fix = 0
    for fn in nc.m.functions:
        for blk in fn.blocks:
            insts = blk.instructions
            i = 0
            while i < len(insts):
                inst = insts[i]
                si = inst.sync_info
                if si is not None and len(si.on_wait) > 1:
                    head, last = si.on_wait[:-1], si.on_wait[-1:]
                    pre = []
                    for w in head:
                        fix += 1
                        nop = mb.InstNoOp(name=f"I-waitfix-{fix}", ins=[],
                                          outs=[])
                        nop.engine = inst.engine
                        nop.sync_info = mb.SyncInfo(on_wait=[w], on_update=[])
                        pre.append(nop)
                    si.on_wait = last
                    insts[i:i] = pre
                    i += len(pre)
                i += 1
    return nc


def _f16(x):
    return np.asarray(x, np.float16)


def _core_inputs(p1, p2, lengths2, core):
    cells = _cells(lengths2)
    movw = CELL * sum(cells)
    inw = QPC + movw
    inp = np.empty((ROWS, inw), np.float16)
    q0 = core * QPB

    for n in range(N):
        a = 2.0 * p1[n, q0:q0 + QPB]       # (1024, 3) f32
        ah = _f16(a)
        al = _f16(a - ah.astype(np.float32))
        sl = slice(n * QPB, (n + 1) * QPB)
        inp[0:3, sl] = ah.T
        inp[3:6, sl] = ah.T
        inp[6:9, sl] = al.T
        inp[9:12, sl] = np.float16(-1.0)

        w = cells[n] * CELL
        b = p2[n, 0:w]                      # (w, 3) f32
        bh = _f16(b)
        bl = _f16(b - bh.astype(np.float32))
        q2 = (b.astype(np.float64) ** 2).sum(1).astype(np.float32)
        q2h = _f16(q2)
        q2l = _f16(q2 - q2h.astype(np.float32))
        mask = np.where(np.arange(w) >= lengths2[n], MASKV, np.float32(0.0))
        off = QPC + CELL * sum(cells[:n])
        ms = slice(off, off + w)
        inp[0:3, ms] = bh.T
        inp[3:6, ms] = bl.T
        inp[6:9, ms] = bh.T
        inp[9, ms] = q2h
        inp[10, ms] = q2l
        inp[11, ms] = _f16(mask)
    return {"inp": inp}


def kernel(p1, p2, lengths1, lengths2):
    from concourse.bass_utils import run_bass_kernel_spmd

    p1 = np.asarray(p1, np.float32)
    p2 = np.asarray(p2, np.float32)
    lengths1 = np.asarray(lengths1, np.int32)
    lengths2 = np.asarray(lengths2, np.int32)

    nc = _build_program(tuple(int(x) for x in lengths2))
    cells = _cells(lengths2)
    in_maps = [_core_inputs(p1, p2, lengths2, c) for c in range(N_CORES)]
    res = run_bass_kernel_spmd(nc, in_maps, core_ids=list(range(N_CORES)))

    # host epilogue: merge candidates, dists = ||p1||^2 - s, pad zeroing
    p1sq = (p1[:, :, 0] * p1[:, :, 0] + p1[:, :, 1] * p1[:, :, 1]) \
        + p1[:, :, 2] * p1[:, :, 2]                      # (4, 8192) f32

    tw = [cells[t // TPB] * 12 for t in range(NTILES)]
    toff = np.concatenate([[0], np.cumsum(tw)]).astype(int)

    dists = np.zeros((N, P1, K), np.float32)
    idx = np.zeros((N, P1, K), np.int64)
    for core in range(N_CORES):
        out = res.results[core]["cand_out"]              # (128, toff[-1]) u32
        for t in range(NTILES):
            n, b = t // TPB, t % TPB
            C = cells[n]
            blk = np.ascontiguousarray(out[:, toff[t]:toff[t + 1]])
            cv = blk[:, 0:C * 8].view(np.float32)        # (128, C*8)
            ci = blk[:, C * 8:C * 12].view(np.uint16)    # (128, C*8)
            # top-16 of C*8 candidates; stable sort = device tie order
            # (value desc, then lower slot = lower cell = lower index)
            slot = np.argsort(-cv, axis=1, kind="stable")[:, :K]
            selV = np.take_along_axis(cv, slot, axis=1)
            rel = np.take_along_axis(ci, slot, axis=1).astype(np.int64)
            rows = slice(core * QPB + b * TILE, core * QPB + (b + 1) * TILE)
            dists[n, rows] = p1sq[n, rows, None] - selV
            idx[n, rows] = (slot >> 3) * CELL + rel

    for n in range(N):
        L = int(lengths1[n])
        dists[n, L:] = 0.0
        idx[n, L:] = 0
    return idx, dists


# revision 30
# speedup vs baseline: 1.1487x; 1.0000x over previous
"""KNN top-16 kernel for Trainium2 (8 NeuronCores, SPMD).

Problem (hardcoded): p1 (4,8192,3) f32, p2 (4,8192,3) f32, lengths1/2 (4,) i32.
Returns (idx int64 (4,8192,16), dists f32 (4,8192,16)) matching
jax.lax.top_k(-sq_dists, 16) semantics with PyTorch3D-style padding.

Sharding (v4 — cross-batch, mask-aware, boundary-trimmed): core c
handles query rows [1024c, 1024(c+1)) of EVERY batch, and scans only
that batch's VALID columns: fully-masked tail cells are skipped
(C_n = ceil(lengths2[n]/1024) cells) and the last cell's scan is
trimmed to its valid prefix. This drops ~14% of the scan work while
keeping all 8 cores exactly balanced. The program is built at
runtime from the actual lengths2 (lru-cached on the lengths tuple).

Device algorithm per 128-query tile (tile t -> batch n = t//8):
  One fp16 matmul per 512-target chunk computes
    s[i,j] = 2*p1_i.p2_j - ||p2_j||^2 - 60000*(j >= len2)
  via a hi/lo fp16 split (12 contraction rows: ah.bh + ah.bl + al.bh
  - q2h - q2l - mask), accurate to ~3e-5 abs.
  DVE reads each PSUM 1024-wide selection cell (2 banks) directly:
    MAX8 -> per-cell top-8 values, FIND_INDEX8 -> relative indices,
  written into one fused u32 candidate block per tile
  (words [0:C*8] = values f32, [C*8:C*12] = indices u16-pairs), which
  ships to host in a single DMA. The host merges C*8 candidates/query
  to top-16 with a stable argsort (reproduces device tie order).
  The DVE thus runs ONLY the two irreducible scans per cell at its
  1 elem/cycle datapath limit. Exactness: the global top-16 is
  covered unless >8 of it falls in one 1024-cell; with fp16-split tie
  noise included, 69 of 512K indices differ from the fp32 reference
  (idx rel-err 9.2e-3, within the 2e-2 gate). Host:
  idx = (slot>>3)*1024 + rel, dists = ||p1||^2 - v.

All per-core inputs are packed into one DRAM tensor, transferred by a
few need-ordered DMAs (TRN2 instructions support at most 2 sync waits
and Tile does not collapse transitive cross-queue waits; the tail
fixup splits any multi-wait onto single-wait NoOps).
"""

import numpy as np
from functools import lru_cache

N, P1, P2, D, K = 4, 8192, 8192, 3, 16
N_CORES = 8
QPB = P1 // N_CORES    # queries per (core, batch) = 1024
QPC = QPB * N          # queries per core (4096)
TILE = 128             # query rows per tile
NTILES = QPC // TILE   # 32 (8 per batch)
TPB = QPB // TILE      # tiles per batch per core (8)
CHUNK = 512            # matmul free-dim chunk == PSUM bank
CELL = 1024            # selection cell (two PSUM banks per MAX8 scan)
ROWS = 12              # fp16 contraction rows
MASKV = np.float32(60000.0)


def _cells(lengths2):
    return tuple(int(-(-int(l) // CELL)) for l in lengths2)


@lru_cache(maxsize=2)
def _build_program(lengths2):
    from concourse.bass import Bass
    from concourse.tile import TileContext
    import concourse.mybir as mybir

    f32 = mybir.dt.float32
    f16 = mybir.dt.float16
    u16 = mybir.dt.uint16
    u32 = mybir.dt.uint32

    cells = _cells(lengths2)
    # scan width per (batch, cell): full 1024 except the last valid cell,
    # which is trimmed to its valid prefix (>=8 for the MAX8 minimum)
    cw = [[min(CELL, max(8, int(lengths2[n]) - c * CELL))
           for c in range(cells[n])] for n in range(N)]
    movw = CELL * sum(cells)        # mov columns (29696 for seed-0 lengths)
    inw = QPC + movw
    moff = [QPC + CELL * sum(cells[:n]) for n in range(N)]  # per-batch mov base
    # fused per-tile candidate block widths (u32 words) and offsets
    tw = [cells[t // TPB] * 12 for t in range(NTILES)]
    toff = np.concatenate([[0], np.cumsum(tw)]).tolist()

    nc = Bass("TRN2", num_devices=N_CORES)

    inp_d = nc.dram_tensor("inp", [ROWS, inw], f16, kind="ExternalInput")
    cand_d = nc.dram_tensor("cand_out", [TILE, toff[-1]], u32,
                            kind="ExternalOutput")

    with TileContext(nc) as tc:
        with tc.tile_pool(name="const", bufs=1) as cpool, \
             tc.tile_pool(name="cand", bufs=2) as spool, \
             tc.tile_pool(name="psum", bufs=4, space="PSUM") as ppool:
            inp_sb = cpool.tile([ROWS, inw], f16)
            # Input DMAs split and ordered by first-need time (same queue =>
            # serial in issue order): tile-0's lhsT, then batch-0 cells 0-2,
            # then the remaining queries, then the rest of the columns.
            def span(a, b):
                nc.sync.dma_start(inp_sb[:, a:b], inp_d[:, a:b])
            span(0, TILE)                                  # stat tile 0
            span(QPC, QPC + CELL)                          # batch-0 cell 0
            span(QPC + CELL, moff[1])                      # batch-0 cells 1+
            span(TILE, QPC)                                # stat tiles 1-31
            span(moff[1], inw)                             # batches 1-3 cells
            stat_sb = inp_sb[:, 0:QPC]

            for t in range(NTILES):
                C = cells[t // TPB]
                cand = spool.tile([TILE, C * 12], u32, tag="cand")
                candV = cand[:, 0:C * 8].bitcast(f32)
                candI = cand[:, C * 8:C * 12].bitcast(u16)
                # 1-element dummy write absorbs the pool slot-reuse wait
                # (the tile's last reader is the Sync-engine DMA) so the
                # real cell ops each carry only the PE-semaphore wait.
                nc.vector.memset(cand[:, 0:1], 0)
                lhsT = stat_sb[:, t * TILE:(t + 1) * TILE]
                base = moff[t // TPB]
                for c in range(C):
                    w = cw[t // TPB][c]
                    ps = ppool.tile([TILE, CELL], f32, tag="ps")
                    for half in range(-(-w // CHUNK)):
                        j0 = base + c * CELL + half * CHUNK
                        hw = min(CHUNK, w - half * CHUNK)
                        nc.tensor.matmul(
                            ps[:, half * CHUNK:half * CHUNK + hw], lhsT,
                            inp_sb[:, j0:j0 + hw],
                            start=True, stop=True,
                        )
                    cv = candV[:, c * 8:(c + 1) * 8]
                    nc.vector.max(out=cv, in_=ps[:, 0:w])
                    nc.vector.max_index(
                        out=candI[:, c * 8:(c + 1) * 8], in_max=cv,
                        in_values=ps[:, 0:w])
                nc.sync.dma_start(cand_d[:, toff[t]:toff[t + 1]], cand)

    # This walrus build allows only ~1 sync wait per instruction; the
    # framework tail Drain carries one wait per busy proc. Split all but
    # the last wait onto single-wait NoOps chained before it (same engine,
    # program order => identical blocking semantics).
    import concourse.mybir as mb
    fix = 0
    for fn in nc.m.functions:
        for blk in fn.blocks:
            insts = blk.instructions
            i = 0
            while i < len(insts):
                inst = insts[i]
                si = inst.sync_info
                if si is not None and len(si.on_wait) > 1:
                    head, last = si.on_wait[:-1], si.on_wait[-1:]
                    pre = []
                    for w in head:
                        fix += 1
                        nop = mb.InstNoOp(name=f"I-waitfix-{fix}", ins=[],
                                          outs=[])
                        nop.engine = inst.engine
                        nop.sync_info = mb.SyncInfo(on_wait=[w], on_update=[])
                        pre.append(nop)
                    si.on_wait = last
                    insts[i:i] = pre
                    i += len(pre)
                i += 1
    return nc


def _f16(x):
    return np.asarray(x, np.float16)


def _core_inputs(p1, p2, lengths2, core):
    cells = _cells(lengths2)
    movw = CELL * sum(cells)
    inw = QPC + movw
    inp = np.empty((ROWS, inw), np.float16)
    q0 = core * QPB

    for n in range(N):
        a = 2.0 * p1[n, q0:q0 + QPB]       # (1024, 3) f32
        ah = _f16(a)
        al = _f16(a - ah.astype(np.float32))
        sl = slice(n * QPB, (n + 1) * QPB)
        inp[0:3, sl] = ah.T
        inp[3:6, sl] = ah.T
        inp[6:9, sl] = al.T
        inp[9:12, sl] = np.float16(-1.0)

        w = cells[n] * CELL
        b = p2[n, 0:w]                      # (w, 3) f32
        bh = _f16(b)
        bl = _f16(b - bh.astype(np.float32))
        q2 = (b.astype(np.float64) ** 2).sum(1).astype(np.float32)
        q2h = _f16(q2)
        q2l = _f16(q2 - q2h.astype(np.float32))
        mask = np.where(np.arange(w) >= lengths2[n], MASKV, np.float32(0.0))
        off = QPC + CELL * sum(cells[:n])
        ms = slice(off, off + w)
        inp[0:3, ms] = bh.T
        inp[3:6, ms] = bl.T
        inp[6:9, ms] = bh.T
        inp[9, ms] = q2h
        inp[10, ms] = q2l
        inp[11, ms] = _f16(mask)
    return {"inp": inp}


def kernel(p1, p2, lengths1, lengths2):
    from concourse.bass_utils import run_bass_kernel_spmd

    p1 = np.asarray(p1, np.float32)
    p2 = np.asarray(p2, np.float32)
    lengths1 = np.asarray(lengths1, np.int32)
    lengths2 = np.asarray(lengths2, np.int32)

    nc = _build_program(tuple(int(x) for x in lengths2))
    cells = _cells(lengths2)
    in_maps = [_core_inputs(p1, p2, lengths2, c) for c in range(N_CORES)]
    res = run_bass_kernel_spmd(nc, in_maps, core_ids=list(range(N_CORES)))

    # host epilogue: merge candidates, dists = ||p1||^2 - s, pad zeroing
    p1sq = (p1[:, :, 0] * p1[:, :, 0] + p1[:, :, 1] * p1[:, :, 1]) \
        + p1[:, :, 2] * p1[:, :, 2]                      # (4, 8192) f32

    tw = [cells[t // TPB] * 12 for t in range(NTILES)]
    toff = np.concatenate([[0], np.cumsum(tw)]).astype(int)

    dists = np.zeros((N, P1, K), np.float32)
    idx = np.zeros((N, P1, K), np.int64)
    for core in range(N_CORES):
        out = res.results[core]["cand_out"]              # (128, toff[-1]) u32
        for t in range(NTILES):
            n, b = t // TPB, t % TPB
            C = cells[n]
            blk = np.ascontiguousarray(out[:, toff[t]:toff[t + 1]])
            cv = blk[:, 0:C * 8].view(np.float32)        # (128, C*8)
            ci = blk[:, C * 8:C * 12].view(np.uint16)    # (128, C*8)
            # top-16 of C*8 candidates; stable sort = device tie order
            # (value desc, then lower slot = lower cell = lower index)
            slot = np.argsort(-cv, axis=1, kind="stable")[:, :K]
            selV = np.take_along_axis(cv, slot, axis=1)
            rel = np.take_along_axis(ci, slot, axis=1).astype(np.int64)
            rows = slice(core * QPB + b * TILE, core * QPB + (b + 1) * TILE)
            dists[n, rows] = p1sq[n, rows, None] - selV
            idx[n, rows] = (slot >> 3) * CELL + rel

    for n in range(N):
        L = int(lengths1[n])
        dists[n, L:] = 0.0
        idx[n, L:] = 0
    return idx, dists


# revision 32
# speedup vs baseline: 1.1497x; 1.0009x over previous
"""KNN top-16 kernel for Trainium2 (8 NeuronCores, SPMD).

Problem (hardcoded): p1 (4,8192,3) f32, p2 (4,8192,3) f32, lengths1/2 (4,) i32.
Returns (idx int64 (4,8192,16), dists f32 (4,8192,16)) matching
jax.lax.top_k(-sq_dists, 16) semantics with PyTorch3D-style padding.

Sharding (v4 — cross-batch, mask-aware, boundary-trimmed): core c
handles query rows [1024c, 1024(c+1)) of EVERY batch, and scans only
that batch's VALID columns: fully-masked tail cells are skipped
(C_n = ceil(lengths2[n]/1024) cells) and the last cell's scan is
trimmed to its valid prefix. This drops ~14% of the scan work while
keeping all 8 cores exactly balanced. The program is built at
runtime from the actual lengths2 (lru-cached on the lengths tuple).

Device algorithm per 128-query tile (tile t -> batch n = t//8):
  One fp16 matmul per 512-target chunk computes
    s[i,j] = 2*p1_i.p2_j - ||p2_j||^2 - 60000*(j >= len2)
  via a hi/lo fp16 split (12 contraction rows: ah.bh + ah.bl + al.bh
  - q2h - q2l - mask), accurate to ~3e-5 abs.
  DVE reads each PSUM 1024-wide selection cell (2 banks) directly:
    MAX8 -> per-cell top-8 values, FIND_INDEX8 -> relative indices,
  written into one fused u32 candidate block per tile
  (words [0:C*8] = values f32, [C*8:C*12] = indices u16-pairs), which
  ships to host in a single DMA. The host merges C*8 candidates/query
  to top-16 with a stable argsort (reproduces device tie order).
  The DVE thus runs ONLY the two irreducible scans per cell at its
  1 elem/cycle datapath limit. Exactness: the global top-16 is
  covered unless >8 of it falls in one 1024-cell; with fp16-split tie
  noise included, 69 of 512K indices differ from the fp32 reference
  (idx rel-err 9.2e-3, within the 2e-2 gate). Host:
  idx = (slot>>3)*1024 + rel, dists = ||p1||^2 - v.

All per-core inputs are packed into one DRAM tensor, transferred by a
few need-ordered DMAs (TRN2 instructions support at most 2 sync waits
and Tile does not collapse transitive cross-queue waits; the tail
fixup splits any multi-wait onto single-wait NoOps).
"""

import numpy as np
from functools import lru_cache

N, P1, P2, D, K = 4, 8192, 8192, 3, 16
N_CORES = 8
QPB = P1 // N_CORES    # queries per (core, batch) = 1024
QPC = QPB * N          # queries per core (4096)
TILE = 128             # query rows per tile
NTILES = QPC // TILE   # 32 (8 per batch)
TPB = QPB // TILE      # tiles per batch per core (8)
CHUNK = 512            # matmul free-dim chunk == PSUM bank
CELL = 1024            # selection cell (two PSUM banks per MAX8 scan)
ROWS = 12              # fp16 contraction rows
MASKV = np.float32(60000.0)


def _cells(lengths2):
    return tuple(int(-(-int(l) // CELL)) for l in lengths2)


@lru_cache(maxsize=2)
def _build_program(lengths2):
    from concourse.bass import Bass
    from concourse.tile import TileContext
    import concourse.mybir as mybir

    f32 = mybir.dt.float32
    f16 = mybir.dt.float16
    u16 = mybir.dt.uint16
    u32 = mybir.dt.uint32

    cells = _cells(lengths2)
    # scan width per (batch, cell): full 1024 except the last valid cell,
    # which is trimmed to its valid prefix (>=8 for the MAX8 minimum)
    cw = [[min(CELL, max(8, int(lengths2[n]) - c * CELL))
           for c in range(cells[n])] for n in range(N)]
    movw = CELL * sum(cells)        # mov columns (29696 for seed-0 lengths)
    inw = QPC + movw
    moff = [QPC + CELL * sum(cells[:n]) for n in range(N)]  # per-batch mov base
    # fused per-tile candidate block widths (u32 words) and offsets
    tw = [cells[t // TPB] * 12 for t in range(NTILES)]
    toff = np.concatenate([[0], np.cumsum(tw)]).tolist()

    nc = Bass("TRN2", num_devices=N_CORES)

    inp_d = nc.dram_tensor("inp", [ROWS, inw], f16, kind="ExternalInput")
    cand_d = nc.dram_tensor("cand_out", [TILE, toff[-1]], u32,
                            kind="ExternalOutput")

    with TileContext(nc) as tc:
        with tc.tile_pool(name="const", bufs=1) as cpool, \
             tc.tile_pool(name="cand", bufs=6) as spool, \
             tc.tile_pool(name="psum", bufs=4, space="PSUM") as ppool:
            inp_sb = cpool.tile([ROWS, inw], f16)
            # Input DMAs split and ordered by first-need time (same queue =>
            # serial in issue order): tile-0's lhsT, then batch-0 cells 0-2,
            # then the remaining queries, then the rest of the columns.
            def span(a, b):
                nc.sync.dma_start(inp_sb[:, a:b], inp_d[:, a:b])
            span(0, TILE)                                  # stat tile 0
            span(QPC, QPC + CELL)                          # batch-0 cell 0
            span(QPC + CELL, QPC + 3 * CELL)               # batch-0 cells 1-2
            span(QPC + 3 * CELL, moff[1])                  # batch-0 cells 3+
            span(TILE, QPC)                                # stat tiles 1-31
            span(moff[1], inw)                             # batches 1-3 cells
            stat_sb = inp_sb[:, 0:QPC]

            for t in range(NTILES):
                C = cells[t // TPB]
                cand = spool.tile([TILE, C * 12], u32, tag="cand")
                candV = cand[:, 0:C * 8].bitcast(f32)
                candI = cand[:, C * 8:C * 12].bitcast(u16)
                # 1-element dummy write absorbs the pool slot-reuse wait
                # (the tile's last reader is the Sync-engine DMA) so the
                # real cell ops each carry only the PE-semaphore wait.
                nc.vector.memset(cand[:, 0:1], 0)
                lhsT = stat_sb[:, t * TILE:(t + 1) * TILE]
                base = moff[t // TPB]
                for c in range(C):
                    w = cw[t // TPB][c]
                    ps = ppool.tile([TILE, CELL], f32, tag="ps")
                    for half in range(-(-w // CHUNK)):
                        j0 = base + c * CELL + half * CHUNK
                        hw = min(CHUNK, w - half * CHUNK)
                        nc.tensor.matmul(
                            ps[:, half * CHUNK:half * CHUNK + hw], lhsT,
                            inp_sb[:, j0:j0 + hw],
                            start=True, stop=True,
                        )
                    cv = candV[:, c * 8:(c + 1) * 8]
                    nc.vector.max(out=cv, in_=ps[:, 0:w])
                    nc.vector.max_index(
                        out=candI[:, c * 8:(c + 1) * 8], in_max=cv,
                        in_values=ps[:, 0:w])
                nc.sync.dma_start(cand_d[:, toff[t]:toff[t + 1]], cand)

    # This walrus build allows only ~1 sync wait per instruction; the
    # framework tail Drain carries one wait per busy proc. Split all but
    # the last wait onto single-wait NoOps chained before it (same engine,
    # program order => identical blocking semantics).
    import concourse.mybir as mb
    fix = 0
    for fn in nc.m.functions:
        for blk in fn.blocks:
            insts = blk.instructions
            i = 0
            while i < len(insts):
                inst = insts[i]
                si = inst.sync_info
                if si is not None and len(si.on_wait) > 1:
                    head, last = si.on_wait[:-1], si.on_wait[-1:]
                    pre = []
                    for w in head:
                        fix += 1
                        nop = mb.InstNoOp(name=f"I-waitfix-{fix}", ins=[],
                                          outs=[])
                        nop.engine = inst.engine
                        nop.sync_info = mb.SyncInfo(on_wait=[w], on_update=[])
                        pre.append(nop)
                    si.on_wait = last
                    insts[i:i] = pre
                    i += len(pre)
                i += 1
    return nc


def _f16(x):
    return np.asarray(x, np.float16)


def _core_inputs(p1, p2, lengths2, core):
    cells = _cells(lengths2)
    movw = CELL * sum(cells)
    inw = QPC + movw
    inp = np.empty((ROWS, inw), np.float16)
    q0 = core * QPB

    for n in range(N):
        a = 2.0 * p1[n, q0:q0 + QPB]       # (1024, 3) f32
        ah = _f16(a)
        al = _f16(a - ah.astype(np.float32))
        sl = slice(n * QPB, (n + 1) * QPB)
        inp[0:3, sl] = ah.T
        inp[3:6, sl] = ah.T
        inp[6:9, sl] = al.T
        inp[9:12, sl] = np.float16(-1.0)

        w = cells[n] * CELL
        b = p2[n, 0:w]                      # (w, 3) f32
        bh = _f16(b)
        bl = _f16(b - bh.astype(np.float32))
        q2 = (b.astype(np.float64) ** 2).sum(1).astype(np.float32)
        q2h = _f16(q2)
        q2l = _f16(q2 - q2h.astype(np.float32))
        mask = np.where(np.arange(w) >= lengths2[n], MASKV, np.float32(0.0))
        off = QPC + CELL * sum(cells[:n])
        ms = slice(off, off + w)
        inp[0:3, ms] = bh.T
        inp[3:6, ms] = bl.T
        inp[6:9, ms] = bh.T
        inp[9, ms] = q2h
        inp[10, ms] = q2l
        inp[11, ms] = _f16(mask)
    return {"inp": inp}


def kernel(p1, p2, lengths1, lengths2):
    from concourse.bass_utils import run_bass_kernel_spmd

    p1 = np.asarray(p1, np.float32)
    p2 = np.asarray(p2, np.float32)
    lengths1 = np.asarray(lengths1, np.int32)
    lengths2 = np.asarray(lengths2, np.int32)

    nc = _build_program(tuple(int(x) for x in lengths2))
    cells = _cells(lengths2)
    in_maps = [_core_inputs(p1, p2, lengths2, c) for c in range(N_CORES)]
    res = run_bass_kernel_spmd(nc, in_maps, core_ids=list(range(N_CORES)))

    # host epilogue: merge candidates, dists = ||p1||^2 - s, pad zeroing
    p1sq = (p1[:, :, 0] * p1[:, :, 0] + p1[:, :, 1] * p1[:, :, 1]) \
        + p1[:, :, 2] * p1[:, :, 2]                      # (4, 8192) f32

    tw = [cells[t // TPB] * 12 for t in range(NTILES)]
    toff = np.concatenate([[0], np.cumsum(tw)]).astype(int)

    dists = np.zeros((N, P1, K), np.float32)
    idx = np.zeros((N, P1, K), np.int64)
    for core in range(N_CORES):
        out = res.results[core]["cand_out"]              # (128, toff[-1]) u32
        for t in range(NTILES):
            n, b = t // TPB, t % TPB
            C = cells[n]
            blk = np.ascontiguousarray(out[:, toff[t]:toff[t + 1]])
            cv = blk[:, 0:C * 8].view(np.float32)        # (128, C*8)
            ci = blk[:, C * 8:C * 12].view(np.uint16)    # (128, C*8)
            # top-16 of C*8 candidates; stable sort = device tie order
            # (value desc, then lower slot = lower cell = lower index)
            slot = np.argsort(-cv, axis=1, kind="stable")[:, :K]
            selV = np.take_along_axis(cv, slot, axis=1)
            rel = np.take_along_axis(ci, slot, axis=1).astype(np.int64)
            rows = slice(core * QPB + b * TILE, core * QPB + (b + 1) * TILE)
            dists[n, rows] = p1sq[n, rows, None] - selV
            idx[n, rows] = (slot >> 3) * CELL + rel

    for n in range(N):
        L = int(lengths1[n])
        dists[n, L:] = 0.0
        idx[n, L:] = 0
    return idx, dists
